# revision 13
# baseline (speedup 1.0000x reference)
"""DiffPool encoder kernel for Trainium2 (Bass/Tile), 8-core SPMD.

Problem (hardcoded shapes):
  S [12288, 10] f32 assignment logits, A [12288, 12288] f32 adjacency,
  X [12288, 300] f32 features, idx [12288] i64 (sorted graph ids),
  n () i64 = 32 nodes/graph. 384 graphs.

  out0 X_cat [3840, 300] = concat_g softmax(S_g)^T X_g
  out1 A_bd [3840, 3840] = block_diag_g softmax(S_g)^T A_g softmax(S_g)

Sharding: graphs split across 8 cores (48 graphs each). Pooling is
block-diagonal per graph, so each core only needs its rows of S/X and the
48 diagonal 32x32 blocks of A. Per core, graphs run in 12 groups of 4
(4*32 = 128 nodes = full partition dim):
  - softmax over K=10 for all 1536 rows (one exp + segmented reduce)
  - SD [128, 12*40]: per group a block-diag [128,40] of normalized S
  - BD [128, 12*128]: per group a block-diag [128,128] of A_b^T
  - mm2: T  = BD_g^T @ Sn_g = A_b @ softmax(S_b), stacked    [128,10]
  - mm1: XO = SD_g^T @ X_g                                   [40,300]
  - mm3: AO = SD_g^T @ T_g                                   [40,10]

The host pre-arranges every input into the exact per-core SBUF layout so
each load is one flat 2D DMA (contiguous per partition; no 40-byte
scatter packets), and post-rearranges the device-layout outputs.

Engine/queue routing: sync HWDGE ring = AT + X chunks 0,2 + all stores;
scalar HWDGE ring = S + X chunks 1,3.  DVE runs BD placement before the
softmax reduce chain so the PE can start mm2s early; PSUM evacuation
alternates ACT/DVE.  Matmul dtype is fp32 by default (exact, 2-pass PE);
set KERNEL_F32R=1 for single-pass float32r (~4x faster mm1, ~2e-4 error).
"""

import os
import numpy as np
from contextlib import ExitStack

B = 384        # graphs
NPER = 32      # nodes per graph
K = 10         # clusters
D = 300        # feature dim
NCORES = 8
GPC = B // NCORES          # 48 graphs per core
GRP = 4                    # graphs per 128-row group
NG = GPC // GRP            # 12 groups per core
ROWS = GPC * NPER          # 1536 node rows per core
XCH = 3                    # groups per X input DMA chunk
OCH = 3                    # groups per XO output DMA chunk

_CACHE = {}
LAST_RESULTS = None        # BassKernelResults of the most recent run


def _use_f32r():
    return bool(os.environ.get("KERNEL_F32R"))


def _body(ctx, tc, S_d, X_d, AT_d, XO_d, AO_d):
    import concourse.bass as bass
    import concourse.mybir as mybir

    nc = tc.nc
    f32 = mybir.dt.float32
    mmdt = mybir.dt.float32r if _use_f32r() else f32

    const = ctx.enter_context(tc.tile_pool(name="const", bufs=1))
    psx = ctx.enter_context(tc.tile_pool(name="psx", bufs=3, space="PSUM"))
    psa = ctx.enter_context(tc.tile_pool(name="psa", bufs=2, space="PSUM"))
    pst = ctx.enter_context(tc.tile_pool(name="pst", bufs=2, space="PSUM"))

    S_t = const.tile([128, NG * K], f32, tag="S_t")
    E = const.tile([128, NG * K], f32, tag="E")
    Sn = const.tile([128, NG * K], mmdt, tag="Sn")
    sums = const.tile([128, NG], f32, tag="sums")
    rinv = const.tile([128, NG], f32, tag="rinv")
    Ast = const.tile([128, NG * NPER], f32, tag="Ast")
    SD = const.tile([128, NG * GRP * K], mmdt, tag="SD")
    BD = const.tile([128, NG * 128], mmdt, tag="BD")
    XT = const.tile([128, NG * (D + K)], mmdt, tag="XT")  # [300 X | 10 T] per group
    XOs = const.tile([GRP * K, NG * D], f32, tag="XOs")
    AOs = const.tile([GRP * K, NG * K], f32, tag="AOs")

    # ---- zero-fills on GpSimd (f32 view), off every critical chain ----
    nc.gpsimd.memset(SD[:].bitcast(f32), 0.0)
    nc.gpsimd.memset(BD[:].bitcast(f32), 0.0)

    # ---- inputs: all flat 2D DMAs (host pre-arranged layouts) ----
    nc.scalar.dma_start(S_t[:], S_d)
    nc.sync.dma_start(Ast[:], AT_d)
    NCH = NG // XCH
    XTv = XT[:].rearrange("p (g m) -> p g m", m=D + K)
    for c in range(NCH):
        (nc.sync if c % 2 == 0 else nc.scalar).dma_start(
            XTv[:, XCH * c:XCH * (c + 1), 0:D],
            X_d[:, XCH * D * c:XCH * D * (c + 1)].rearrange(
                "p (g d) -> p g d", d=D))

    # ---- block-diag A^T placement on DVE first (only needs AT) ----
    A3 = Ast[:].rearrange("p (g q) -> p g q", q=NPER)
    BDv = BD[:].rearrange("p (g m) -> p g m", m=128)
    for b in range(GRP):
        ps = slice(NPER * b, NPER * (b + 1))
        eng = nc.vector.tensor_copy if b % 2 == 0 else nc.scalar.copy
        eng(BDv[ps, :, NPER * b:NPER * (b + 1)], A3[ps, :, :])

    # ---- softmax over K within each group column block ----
    nc.scalar.activation(E[:], S_t[:], mybir.ActivationFunctionType.Exp)
    E3 = E[:].rearrange("p (g k) -> p g k", k=K)
    nc.vector.reduce_sum(sums[:], E3, axis=mybir.AxisListType.X)
    nc.vector.reciprocal(rinv[:], sums[:])
    rb = rinv[:].unsqueeze(2)
    nc.vector.tensor_mul(Sn[:].rearrange("p (g k) -> p g k", k=K), E3,
                         rb.broadcast_to([128, NG, K]))

    # ---- block-diag softmax placement on DVE ----
    Sn3 = Sn[:].rearrange("p (g k) -> p g k", k=K)
    SDv = SD[:].rearrange("p (g m) -> p g m", m=GRP * K)
    for b in range(GRP):
        ps = slice(NPER * b, NPER * (b + 1))
        nc.vector.tensor_copy(SDv[ps, :, K * b:K * (b + 1)], Sn3[ps, :, :])

    # ---- loop A: all mm2 first (PE can start as soon as BD+Sn land) ----
    W = D + K
    for g in range(NG):
        tp = pst.tile([128, K], f32)
        nc.tensor.matmul(tp[:], BD[:, 128 * g:128 * (g + 1)],
                         Sn[:, K * g:K * (g + 1)], start=True, stop=True)
        nc.scalar.copy(XT[:, W * g + D:W * (g + 1)], tp[:])

    # ---- loop B: mm1 + mm3 per group, stores on the sync ring ----
    for g in range(NG):
        def cp_a(out, in_, even=(g % 2 == 0)):
            (nc.scalar.copy if even else nc.vector.tensor_copy)(out, in_)

        def cp_b(out, in_, even=(g % 2 == 0)):
            (nc.vector.tensor_copy if even else nc.scalar.copy)(out, in_)

        xo = psx.tile([GRP * K, W], f32)
        nc.tensor.matmul(xo[:], SD[:, GRP * K * g:GRP * K * (g + 1)],
                         XT[:, W * g:W * (g + 1)], start=True, stop=True)
        cp_a(XOs[:, D * g:D * (g + 1)], xo[:, 0:D])
        cp_b(AOs[:, K * g:K * (g + 1)], xo[:, D:W])

        # stores: coarse early chunks, fine late ones, split across rings
        if g in (2, 5, 8):
            c = g // OCH
            nc.sync.dma_start(XO_d[:, D * OCH * c:D * OCH * (c + 1)],
                              XOs[:, D * OCH * c:D * OCH * (c + 1)])
        elif g >= 9:
            ring = nc.sync if g % 2 == 1 else nc.scalar
            ring.dma_start(XO_d[:, D * g:D * (g + 1)],
                           XOs[:, D * g:D * (g + 1)])
        if g in (NG // 2 - 1, NG - 1):
            h = 0 if g == NG // 2 - 1 else 1
            hw = NG // 2 * K
            (nc.scalar if h else nc.sync).dma_start(
                AO_d[:, hw * h:hw * (h + 1)], AOs[:, hw * h:hw * (h + 1)])


def _build():
    key = ("nc", _use_f32r())
    if key in _CACHE:
        return _CACHE[key]
    import concourse.bacc as bacc
    import concourse.tile as tile
    import concourse.mybir as mybir

    f32 = mybir.dt.float32
    mmdt = mybir.dt.float32r if _use_f32r() else f32
    nc = bacc.Bacc("TRN2", target_bir_lowering=False, debug=False)
    # Device-layout tensors (host pre/post-arranges):
    #   S  [128, 120]   col = 10g + k, partition = node p of group g
    #   AT [128, 384]   [32b+q, 32g+p] = A_{4g+b}[p, q]
    #   X  [128, 3600]  col = 300g + d
    #   XO [40, 3600]   row = 10b + i, col = 300g + d  (graph j = 4g+b)
    #   AO [40, 120]    row = 10b + i, col = 10g + k
    S_d = nc.dram_tensor("S", [128, NG * K], f32, kind="ExternalInput").ap()
    X_d = nc.dram_tensor("X", [128, NG * D], mmdt, kind="ExternalInput").ap()
    AT_d = nc.dram_tensor("AT", [128, NG * NPER], f32, kind="ExternalInput").ap()
    XO_d = nc.dram_tensor("XO", [GRP * K, NG * D], f32, kind="ExternalOutput").ap()
    AO_d = nc.dram_tensor("AO", [GRP * K, NG * K], f32, kind="ExternalOutput").ap()

    with tile.TileContext(nc) as tc:
        with ExitStack() as ctx:
            _body(ctx, tc, S_d, X_d, AT_d, XO_d, AO_d)
    nc.compile()
    _CACHE[key] = nc
    return nc


def kernel(S, A, X, idx=None, n=NPER, **_):
    global LAST_RESULTS
    from concourse.bass_utils import run_bass_kernel_spmd

    S = np.asarray(S, dtype=np.float32)
    A = np.asarray(A, dtype=np.float32)
    X = np.asarray(X, dtype=np.float32)
    n = int(np.asarray(n)) if n is not None else NPER
    assert n == NPER and S.shape == (B * NPER, K) and X.shape == (B * NPER, D)

    # Device layouts (see _build).  c = core, g = group, b = graph-in-group.
    S8 = np.ascontiguousarray(
        S.reshape(NCORES, NG, 128, K).transpose(0, 2, 1, 3)
    ).reshape(NCORES, 128, NG * K)
    X8 = np.ascontiguousarray(
        X.reshape(NCORES, NG, 128, D).transpose(0, 2, 1, 3)
    ).reshape(NCORES, 128, NG * D)
    bi = np.arange(B)
    blocks = A.reshape(B, NPER, B, NPER)[bi, :, bi, :]        # [384, 32, 32]
    blocksT = blocks.transpose(0, 2, 1)                       # [j][q, p] = A_j[p, q]
    AT8 = np.ascontiguousarray(
        blocksT.reshape(NCORES, NG, GRP, NPER, NPER).transpose(0, 2, 3, 1, 4)
    ).reshape(NCORES, 128, NG * NPER)

    in_maps = [{"S": S8[c], "X": X8[c], "AT": AT8[c]} for c in range(NCORES)]

    nc = _build()
    kw = {}
    if os.environ.get("KERNEL_TRACE"):
        kw = dict(trace=True, tmpdir=os.environ.get("KERNEL_TRACE_DIR") or None)
    res = run_bass_kernel_spmd(nc, in_maps, list(range(NCORES)), **kw)
    LAST_RESULTS = res

    # XO [40, 3600] -> per-core [12, 40, 300] -> rows 40g+10b+i of X_cat
    X_cat = np.concatenate(
        [r["XO"].reshape(GRP * K, NG, D).transpose(1, 0, 2).reshape(GPC * K, D)
         for r in res.results], axis=0)
    # AO [40, 120] -> blocks [g, b][i, k] -> graph j = 4g+b
    AO = np.stack(
        [r["AO"].reshape(GRP, K, NG, K).transpose(2, 0, 1, 3).reshape(GPC, K, K)
         for r in res.results]).reshape(B, K, K)
    A_bd = np.zeros((B * K, B * K), dtype=np.float32)
    A_bd.reshape(B, K, B, K)[bi, :, bi, :] = AO
    return X_cat, A_bd


# revision 14
# speedup vs baseline: 1.0288x; 1.0288x over previous
"""DiffPool encoder kernel for Trainium2 (Bass/Tile), 8-core SPMD.

Problem (hardcoded shapes):
  S [12288, 10] f32 assignment logits, A [12288, 12288] f32 adjacency,
  X [12288, 300] f32 features, idx [12288] i64 (sorted graph ids),
  n () i64 = 32 nodes/graph. 384 graphs.

  out0 X_cat [3840, 300] = concat_g softmax(S_g)^T X_g
  out1 A_bd [3840, 3840] = block_diag_g softmax(S_g)^T A_g softmax(S_g)

Sharding: graphs split across 8 cores (48 graphs each). Pooling is
block-diagonal per graph, so each core only needs its rows of S/X and the
48 diagonal 32x32 blocks of A. Per core, graphs run in 12 groups of 4
(4*32 = 128 nodes = full partition dim):
  - softmax over K=10 for all 1536 rows (one exp + segmented reduce)
  - SD [128, 12*40]: per group a block-diag [128,40] of normalized S
  - BD [128, 12*128]: per group a block-diag [128,128] of A_b^T
  - mm2: T  = BD_g^T @ Sn_g = A_b @ softmax(S_b), stacked    [128,10]
  - mm1: XO = SD_g^T @ X_g                                   [40,300]
  - mm3: AO = SD_g^T @ T_g                                   [40,10]

The host pre-arranges every input into the exact per-core SBUF layout so
each load is one flat 2D DMA (contiguous per partition; no 40-byte
scatter packets), and post-rearranges the device-layout outputs.

Engine/queue routing: sync HWDGE ring = AT + X chunks 0,2 + all stores;
scalar HWDGE ring = S + X chunks 1,3.  DVE runs BD placement before the
softmax reduce chain so the PE can start mm2s early; PSUM evacuation
alternates ACT/DVE.  Matmul dtype is fp32 by default (exact, 2-pass PE);
set KERNEL_F32R=1 for single-pass float32r (~4x faster mm1, ~2e-4 error).
"""

import os
import numpy as np
from contextlib import ExitStack

B = 384        # graphs
NPER = 32      # nodes per graph
K = 10         # clusters
D = 300        # feature dim
NCORES = 8
GPC = B // NCORES          # 48 graphs per core
GRP = 4                    # graphs per 128-row group
NG = GPC // GRP            # 12 groups per core
ROWS = GPC * NPER          # 1536 node rows per core
XCH = 3                    # groups per X input DMA chunk
OCH = 3                    # groups per XO output DMA chunk

_CACHE = {}
LAST_RESULTS = None        # BassKernelResults of the most recent run


def _use_f32r():
    return bool(os.environ.get("KERNEL_F32R"))


def _body(ctx, tc, S_d, X_d, AT_d, XO_d, AO_d):
    import concourse.bass as bass
    import concourse.mybir as mybir

    nc = tc.nc
    f32 = mybir.dt.float32
    mmdt = mybir.dt.float32r if _use_f32r() else f32

    const = ctx.enter_context(tc.tile_pool(name="const", bufs=1))
    psx = ctx.enter_context(tc.tile_pool(name="psx", bufs=3, space="PSUM"))
    psa = ctx.enter_context(tc.tile_pool(name="psa", bufs=2, space="PSUM"))
    pst = ctx.enter_context(tc.tile_pool(name="pst", bufs=2, space="PSUM"))

    S_t = const.tile([128, NG * K], f32, tag="S_t")
    E = const.tile([128, NG * K], f32, tag="E")
    Sn = const.tile([128, NG * K], mmdt, tag="Sn")
    sums = const.tile([128, NG], f32, tag="sums")
    rinv = const.tile([128, NG], f32, tag="rinv")
    Ast = const.tile([128, NG * NPER], f32, tag="Ast")
    SD = const.tile([128, NG * GRP * K], mmdt, tag="SD")
    BD = const.tile([128, NG * 128], mmdt, tag="BD")
    XT = const.tile([128, NG * (D + K)], mmdt, tag="XT")  # [300 X | 10 T] per group
    XOs = const.tile([GRP * K, NG * D], f32, tag="XOs")
    AOs = const.tile([GRP * K, NG * K], f32, tag="AOs")

    # ---- zero-fills on GpSimd (f32 view), off every critical chain ----
    nc.gpsimd.memset(SD[:].bitcast(f32), 0.0)
    nc.gpsimd.memset(BD[:].bitcast(f32), 0.0)

    # ---- inputs: all flat 2D DMAs (host pre-arranged layouts) ----
    nc.scalar.dma_start(S_t[:], S_d)
    nc.sync.dma_start(Ast[:], AT_d)
    NCH = NG // XCH
    XTv = XT[:].rearrange("p (g m) -> p g m", m=D + K)
    for c in range(NCH):
        (nc.sync if c % 2 == 0 else nc.scalar).dma_start(
            XTv[:, XCH * c:XCH * (c + 1), 0:D],
            X_d[:, XCH * D * c:XCH * D * (c + 1)].rearrange(
                "p (g d) -> p g d", d=D))

    # ---- block-diag A^T placement on DVE first (only needs AT) ----
    A3 = Ast[:].rearrange("p (g q) -> p g q", q=NPER)
    BDv = BD[:].rearrange("p (g m) -> p g m", m=128)
    for b in range(GRP):
        ps = slice(NPER * b, NPER * (b + 1))
        eng = nc.vector.tensor_copy if b % 2 == 0 else nc.scalar.copy
        eng(BDv[ps, :, NPER * b:NPER * (b + 1)], A3[ps, :, :])

    # ---- softmax over K within each group column block ----
    nc.scalar.activation(E[:], S_t[:], mybir.ActivationFunctionType.Exp)
    E3 = E[:].rearrange("p (g k) -> p g k", k=K)
    nc.vector.reduce_sum(sums[:], E3, axis=mybir.AxisListType.X)
    nc.vector.reciprocal(rinv[:], sums[:])
    rb = rinv[:].unsqueeze(2)
    nc.vector.tensor_mul(Sn[:].rearrange("p (g k) -> p g k", k=K), E3,
                         rb.broadcast_to([128, NG, K]))

    # ---- block-diag softmax placement on DVE ----
    Sn3 = Sn[:].rearrange("p (g k) -> p g k", k=K)
    SDv = SD[:].rearrange("p (g m) -> p g m", m=GRP * K)
    for b in range(GRP):
        ps = slice(NPER * b, NPER * (b + 1))
        nc.vector.tensor_copy(SDv[ps, :, K * b:K * (b + 1)], Sn3[ps, :, :])

    # ---- loop A: all mm2 first (PE can start as soon as BD+Sn land) ----
    W = D + K
    for g in range(NG):
        tp = pst.tile([128, K], f32)
        nc.tensor.matmul(tp[:], BD[:, 128 * g:128 * (g + 1)],
                         Sn[:, K * g:K * (g + 1)], start=True, stop=True)
        nc.scalar.copy(XT[:, W * g + D:W * (g + 1)], tp[:])

    # ---- loop B: merged mm1+mm3 per group; trailing stores kept small ----
    for g in range(NG):
        # PSUM evacuation: late groups pinned (XO->ACT, AO->DVE) so the
        # final copies never queue behind store DMAs on one engine
        if g >= 10:
            cp_xo, cp_ao = nc.scalar.copy, nc.vector.tensor_copy
        elif g % 2 == 0:
            cp_xo, cp_ao = nc.scalar.copy, nc.vector.tensor_copy
        else:
            cp_xo, cp_ao = nc.vector.tensor_copy, nc.scalar.copy

        xo = psx.tile([GRP * K, W], f32)
        nc.tensor.matmul(xo[:], SD[:, GRP * K * g:GRP * K * (g + 1)],
                         XT[:, W * g:W * (g + 1)], start=True, stop=True)
        cp_xo(XOs[:, D * g:D * (g + 1)], xo[:, 0:D])
        cp_ao(AOs[:, K * g:K * (g + 1)], xo[:, D:W])

        # stores: coarse early chunks, fine late ones, all on the sync ring
        # (scalar engine is busy with the trailing copies)
        if g in (2, 5, 8):
            c = g // OCH
            nc.sync.dma_start(XO_d[:, D * OCH * c:D * OCH * (c + 1)],
                              XOs[:, D * OCH * c:D * OCH * (c + 1)])
        elif g >= 9:
            nc.sync.dma_start(XO_d[:, D * g:D * (g + 1)],
                              XOs[:, D * g:D * (g + 1)])
        if g in (NG // 2 - 1, NG - 1):
            h = 0 if g == NG // 2 - 1 else 1
            hw = NG // 2 * K
            nc.sync.dma_start(
                AO_d[:, hw * h:hw * (h + 1)], AOs[:, hw * h:hw * (h + 1)])


def _build():
    key = ("nc", _use_f32r())
    if key in _CACHE:
        return _CACHE[key]
    import concourse.bacc as bacc
    import concourse.tile as tile
    import concourse.mybir as mybir

    f32 = mybir.dt.float32
    mmdt = mybir.dt.float32r if _use_f32r() else f32
    nc = bacc.Bacc("TRN2", target_bir_lowering=False, debug=False)
    # Device-layout tensors (host pre/post-arranges):
    #   S  [128, 120]   col = 10g + k, partition = node p of group g
    #   AT [128, 384]   [32b+q, 32g+p] = A_{4g+b}[p, q]
    #   X  [128, 3600]  col = 300g + d
    #   XO [40, 3600]   row = 10b + i, col = 300g + d  (graph j = 4g+b)
    #   AO [40, 120]    row = 10b + i, col = 10g + k
    S_d = nc.dram_tensor("S", [128, NG * K], f32, kind="ExternalInput").ap()
    X_d = nc.dram_tensor("X", [128, NG * D], mmdt, kind="ExternalInput").ap()
    AT_d = nc.dram_tensor("AT", [128, NG * NPER], f32, kind="ExternalInput").ap()
    XO_d = nc.dram_tensor("XO", [GRP * K, NG * D], f32, kind="ExternalOutput").ap()
    AO_d = nc.dram_tensor("AO", [GRP * K, NG * K], f32, kind="ExternalOutput").ap()

    with tile.TileContext(nc) as tc:
        with ExitStack() as ctx:
            _body(ctx, tc, S_d, X_d, AT_d, XO_d, AO_d)
    nc.compile()
    _CACHE[key] = nc
    return nc


def kernel(S, A, X, idx=None, n=NPER, **_):
    global LAST_RESULTS
    from concourse.bass_utils import run_bass_kernel_spmd

    S = np.asarray(S, dtype=np.float32)
    A = np.asarray(A, dtype=np.float32)
    X = np.asarray(X, dtype=np.float32)
    n = int(np.asarray(n)) if n is not None else NPER
    assert n == NPER and S.shape == (B * NPER, K) and X.shape == (B * NPER, D)

    # Device layouts (see _build).  c = core, g = group, b = graph-in-group.
    S8 = np.ascontiguousarray(
        S.reshape(NCORES, NG, 128, K).transpose(0, 2, 1, 3)
    ).reshape(NCORES, 128, NG * K)
    X8 = np.ascontiguousarray(
        X.reshape(NCORES, NG, 128, D).transpose(0, 2, 1, 3)
    ).reshape(NCORES, 128, NG * D)
    bi = np.arange(B)
    blocks = A.reshape(B, NPER, B, NPER)[bi, :, bi, :]        # [384, 32, 32]
    blocksT = blocks.transpose(0, 2, 1)                       # [j][q, p] = A_j[p, q]
    AT8 = np.ascontiguousarray(
        blocksT.reshape(NCORES, NG, GRP, NPER, NPER).transpose(0, 2, 3, 1, 4)
    ).reshape(NCORES, 128, NG * NPER)

    in_maps = [{"S": S8[c], "X": X8[c], "AT": AT8[c]} for c in range(NCORES)]

    nc = _build()
    kw = {}
    if os.environ.get("KERNEL_TRACE"):
        kw = dict(trace=True, tmpdir=os.environ.get("KERNEL_TRACE_DIR") or None)
    res = run_bass_kernel_spmd(nc, in_maps, list(range(NCORES)), **kw)
    LAST_RESULTS = res

    # XO [40, 3600] -> per-core [12, 40, 300] -> rows 40g+10b+i of X_cat
    X_cat = np.concatenate(
        [r["XO"].reshape(GRP * K, NG, D).transpose(1, 0, 2).reshape(GPC * K, D)
         for r in res.results], axis=0)
    # AO [40, 120] -> blocks [g, b][i, k] -> graph j = 4g+b
    AO = np.stack(
        [r["AO"].reshape(GRP, K, NG, K).transpose(2, 0, 1, 3).reshape(GPC, K, K)
         for r in res.results]).reshape(B, K, K)
    A_bd = np.zeros((B * K, B * K), dtype=np.float32)
    A_bd.reshape(B, K, B, K)[bi, :, bi, :] = AO
    return X_cat, A_bd


# revision 15
# speedup vs baseline: 1.2099x; 1.1760x over previous
"""DiffPool encoder kernel for Trainium2 (Bass/Tile), 8-core SPMD.

Problem (hardcoded shapes):
  S [12288, 10] f32 assignment logits, A [12288, 12288] f32 adjacency,
  X [12288, 300] f32 features, idx [12288] i64 (sorted graph ids),
  n () i64 = 32 nodes/graph. 384 graphs.

  out0 X_cat [3840, 300] = concat_g softmax(S_g)^T X_g
  out1 A_bd [3840, 3840] = block_diag_g softmax(S_g)^T A_g softmax(S_g)

Sharding: graphs split across 8 cores (48 graphs each). Pooling is
block-diagonal per graph, so each core only needs its rows of S/X and the
48 diagonal 32x32 blocks of A. Per core, graphs run in 12 groups of 4
(4*32 = 128 nodes = full partition dim):
  - softmax over K=10 for all 1536 rows (one exp + segmented reduce)
  - SD [128, 12*40]: per group a block-diag [128,40] of normalized S
  - BD [128, 12*128]: per group a block-diag [128,128] of A_b^T
  - mm2: T  = BD_g^T @ Sn_g = A_b @ softmax(S_b), stacked    [128,10]
  - mm1: XO = SD_g^T @ X_g                                   [40,300]
  - mm3: AO = SD_g^T @ T_g                                   [40,10]

The host pre-arranges every input into the exact per-core SBUF layout so
each load is one flat 2D DMA (contiguous per partition; no 40-byte
scatter packets), and post-rearranges the device-layout outputs.

Engine/queue routing: sync HWDGE ring = AT + X chunks 0,2 + all stores;
scalar HWDGE ring = S + X chunks 1,3.  DVE runs BD placement before the
softmax reduce chain so the PE can start mm2s early; PSUM evacuation
alternates ACT/DVE.  Matmul dtype is fp32 by default (exact, 2-pass PE);
set KERNEL_F32R=1 for single-pass float32r (~4x faster mm1, ~2e-4 error).
"""

import os
import numpy as np
from contextlib import ExitStack

B = 384        # graphs
NPER = 32      # nodes per graph
K = 10         # clusters
D = 300        # feature dim
NCORES = 8
GPC = B // NCORES          # 48 graphs per core
GRP = 4                    # graphs per 128-row group
NG = GPC // GRP            # 12 groups per core
ROWS = GPC * NPER          # 1536 node rows per core
XCH = 3                    # groups per X input DMA chunk
OCH = 3                    # groups per XO output DMA chunk

_CACHE = {}
LAST_RESULTS = None        # BassKernelResults of the most recent run


def _use_f32r():
    return bool(os.environ.get("KERNEL_F32R"))


def _body(ctx, tc, S_d, X_d, AT_d, XO_d, AO_d):
    import concourse.bass as bass
    import concourse.mybir as mybir

    nc = tc.nc
    f32 = mybir.dt.float32
    mmdt = mybir.dt.float32r if _use_f32r() else f32

    const = ctx.enter_context(tc.tile_pool(name="const", bufs=1))
    psx = ctx.enter_context(tc.tile_pool(name="psx", bufs=3, space="PSUM"))
    psa = ctx.enter_context(tc.tile_pool(name="psa", bufs=2, space="PSUM"))
    pst = ctx.enter_context(tc.tile_pool(name="pst", bufs=2, space="PSUM"))

    S_t = const.tile([128, NG * K], f32, tag="S_t")
    E = const.tile([128, NG * K], f32, tag="E")
    Sn = const.tile([128, NG * K], mmdt, tag="Sn")
    bf16 = mybir.dt.bfloat16
    Snh = const.tile([128, NG * K], bf16, tag="Snh")   # bf16 hi/lo pair of Sn
    Snl = const.tile([128, NG * K], bf16, tag="Snl")
    sums = const.tile([128, NG], f32, tag="sums")
    rinv = const.tile([128, NG], f32, tag="rinv")
    Ast = const.tile([128, NG * NPER], f32, tag="Ast")
    SD = const.tile([128, NG * GRP * K], mmdt, tag="SD")
    BD = const.tile([128, NG * 128], mybir.dt.bfloat16, tag="BD")  # 0/1: exact
    XT = const.tile([128, NG * (D + K)], mmdt, tag="XT")  # [300 X | 10 T] per group
    XOs = const.tile([GRP * K, NG * D], f32, tag="XOs")
    AOs = const.tile([GRP * K, NG * K], f32, tag="AOs")

    # ---- zero-fills on GpSimd (f32 view), off every critical chain ----
    nc.gpsimd.memset(SD[:].bitcast(f32), 0.0)
    nc.gpsimd.memset(BD[:], 0.0)

    # ---- inputs: all flat 2D DMAs (host pre-arranged layouts) ----
    nc.scalar.dma_start(S_t[:], S_d)
    nc.sync.dma_start(Ast[:], AT_d)
    NCH = NG // XCH
    XTv = XT[:].rearrange("p (g m) -> p g m", m=D + K)
    for c in range(NCH):
        (nc.sync if c % 2 == 0 else nc.scalar).dma_start(
            XTv[:, XCH * c:XCH * (c + 1), 0:D],
            X_d[:, XCH * D * c:XCH * D * (c + 1)].rearrange(
                "p (g d) -> p g d", d=D))

    # ---- block-diag A^T placement on DVE first (only needs AT) ----
    A3 = Ast[:].rearrange("p (g q) -> p g q", q=NPER)
    BDv = BD[:].rearrange("p (g m) -> p g m", m=128)
    for b in range(GRP):
        ps = slice(NPER * b, NPER * (b + 1))
        eng = nc.vector.tensor_copy if b % 2 == 0 else nc.scalar.copy
        eng(BDv[ps, :, NPER * b:NPER * (b + 1)], A3[ps, :, :])

    # ---- softmax over K within each group column block ----
    nc.scalar.activation(E[:], S_t[:], mybir.ActivationFunctionType.Exp)
    E3 = E[:].rearrange("p (g k) -> p g k", k=K)
    nc.vector.reduce_sum(sums[:], E3, axis=mybir.AxisListType.X)
    nc.vector.reciprocal(rinv[:], sums[:])
    rb = rinv[:].unsqueeze(2)
    nc.vector.tensor_mul(Sn[:].rearrange("p (g k) -> p g k", k=K), E3,
                         rb.broadcast_to([128, NG, K]))

    nc.vector.tensor_copy(Snh[:], Sn[:])
    nc.vector.tensor_sub(Snl[:], Sn[:], Snh[:])

    # ---- block-diag softmax placement on DVE ----
    Sn3 = Sn[:].rearrange("p (g k) -> p g k", k=K)
    SDv = SD[:].rearrange("p (g m) -> p g m", m=GRP * K)
    for b in range(GRP):
        ps = slice(NPER * b, NPER * (b + 1))
        nc.vector.tensor_copy(SDv[ps, :, K * b:K * (b + 1)], Sn3[ps, :, :])

    # ---- loop A: all mm2 first (PE can start as soon as BD+Sn land) ----
    W = D + K
    for g in range(NG):
        tp = pst.tile([128, K], f32)
        nc.tensor.matmul(tp[:], BD[:, 128 * g:128 * (g + 1)],
                         Snh[:, K * g:K * (g + 1)], start=True, stop=False)
        nc.tensor.matmul(tp[:], BD[:, 128 * g:128 * (g + 1)],
                         Snl[:, K * g:K * (g + 1)], start=False, stop=True)
        nc.scalar.copy(XT[:, W * g + D:W * (g + 1)], tp[:])

    # ---- loop B: merged mm1+mm3 per group; trailing stores kept small ----
    for g in range(NG):
        # PSUM evacuation: late groups pinned (XO->ACT, AO->DVE) so the
        # final copies never queue behind store DMAs on one engine
        if g >= 10:
            cp_xo, cp_ao = nc.scalar.copy, nc.vector.tensor_copy
        elif g % 2 == 0:
            cp_xo, cp_ao = nc.scalar.copy, nc.vector.tensor_copy
        else:
            cp_xo, cp_ao = nc.vector.tensor_copy, nc.scalar.copy

        xo = psx.tile([GRP * K, W], f32)
        nc.tensor.matmul(xo[:], SD[:, GRP * K * g:GRP * K * (g + 1)],
                         XT[:, W * g:W * (g + 1)], start=True, stop=True)
        cp_xo(XOs[:, D * g:D * (g + 1)], xo[:, 0:D])
        cp_ao(AOs[:, K * g:K * (g + 1)], xo[:, D:W])

        # stores: coarse early chunks, fine late ones, all on the sync ring
        # (scalar engine is busy with the trailing copies)
        if g in (2, 5, 8):
            c = g // OCH
            nc.sync.dma_start(XO_d[:, D * OCH * c:D * OCH * (c + 1)],
                              XOs[:, D * OCH * c:D * OCH * (c + 1)])
        elif g >= 9:
            nc.sync.dma_start(XO_d[:, D * g:D * (g + 1)],
                              XOs[:, D * g:D * (g + 1)])
        if g in (NG // 2 - 1, NG - 1):
            h = 0 if g == NG // 2 - 1 else 1
            hw = NG // 2 * K
            nc.sync.dma_start(
                AO_d[:, hw * h:hw * (h + 1)], AOs[:, hw * h:hw * (h + 1)])


def _build():
    key = ("nc", _use_f32r())
    if key in _CACHE:
        return _CACHE[key]
    import concourse.bacc as bacc
    import concourse.tile as tile
    import concourse.mybir as mybir

    f32 = mybir.dt.float32
    mmdt = mybir.dt.float32r if _use_f32r() else f32
    nc = bacc.Bacc("TRN2", target_bir_lowering=False, debug=False)
    # Device-layout tensors (host pre/post-arranges):
    #   S  [128, 120]   col = 10g + k, partition = node p of group g
    #   AT [128, 384]   [32b+q, 32g+p] = A_{4g+b}[p, q]
    #   X  [128, 3600]  col = 300g + d
    #   XO [40, 3600]   row = 10b + i, col = 300g + d  (graph j = 4g+b)
    #   AO [40, 120]    row = 10b + i, col = 10g + k
    S_d = nc.dram_tensor("S", [128, NG * K], f32, kind="ExternalInput").ap()
    X_d = nc.dram_tensor("X", [128, NG * D], mmdt, kind="ExternalInput").ap()
    AT_d = nc.dram_tensor("AT", [128, NG * NPER], f32, kind="ExternalInput").ap()
    XO_d = nc.dram_tensor("XO", [GRP * K, NG * D], f32, kind="ExternalOutput").ap()
    AO_d = nc.dram_tensor("AO", [GRP * K, NG * K], f32, kind="ExternalOutput").ap()

    with tile.TileContext(nc) as tc:
        with ExitStack() as ctx:
            _body(ctx, tc, S_d, X_d, AT_d, XO_d, AO_d)
    nc.compile()
    _CACHE[key] = nc
    return nc


def kernel(S, A, X, idx=None, n=NPER, **_):
    global LAST_RESULTS
    from concourse.bass_utils import run_bass_kernel_spmd

    S = np.asarray(S, dtype=np.float32)
    A = np.asarray(A, dtype=np.float32)
    X = np.asarray(X, dtype=np.float32)
    n = int(np.asarray(n)) if n is not None else NPER
    assert n == NPER and S.shape == (B * NPER, K) and X.shape == (B * NPER, D)

    # Device layouts (see _build).  c = core, g = group, b = graph-in-group.
    S8 = np.ascontiguousarray(
        S.reshape(NCORES, NG, 128, K).transpose(0, 2, 1, 3)
    ).reshape(NCORES, 128, NG * K)
    X8 = np.ascontiguousarray(
        X.reshape(NCORES, NG, 128, D).transpose(0, 2, 1, 3)
    ).reshape(NCORES, 128, NG * D)
    bi = np.arange(B)
    blocks = A.reshape(B, NPER, B, NPER)[bi, :, bi, :]        # [384, 32, 32]
    blocksT = blocks.transpose(0, 2, 1)                       # [j][q, p] = A_j[p, q]
    AT8 = np.ascontiguousarray(
        blocksT.reshape(NCORES, NG, GRP, NPER, NPER).transpose(0, 2, 3, 1, 4)
    ).reshape(NCORES, 128, NG * NPER)

    in_maps = [{"S": S8[c], "X": X8[c], "AT": AT8[c]} for c in range(NCORES)]

    nc = _build()
    kw = {}
    if os.environ.get("KERNEL_TRACE"):
        kw = dict(trace=True, tmpdir=os.environ.get("KERNEL_TRACE_DIR") or None)
    res = run_bass_kernel_spmd(nc, in_maps, list(range(NCORES)), **kw)
    LAST_RESULTS = res

    # XO [40, 3600] -> per-core [12, 40, 300] -> rows 40g+10b+i of X_cat
    X_cat = np.concatenate(
        [r["XO"].reshape(GRP * K, NG, D).transpose(1, 0, 2).reshape(GPC * K, D)
         for r in res.results], axis=0)
    # AO [40, 120] -> blocks [g, b][i, k] -> graph j = 4g+b
    AO = np.stack(
        [r["AO"].reshape(GRP, K, NG, K).transpose(2, 0, 1, 3).reshape(GPC, K, K)
         for r in res.results]).reshape(B, K, K)
    A_bd = np.zeros((B * K, B * K), dtype=np.float32)
    A_bd.reshape(B, K, B, K)[bi, :, bi, :] = AO
    return X_cat, A_bd


# revision 16
# speedup vs baseline: 1.2338x; 1.0197x over previous
"""DiffPool encoder kernel for Trainium2 (Bass/Tile), 8-core SPMD.

Problem (hardcoded shapes):
  S [12288, 10] f32 assignment logits, A [12288, 12288] f32 adjacency,
  X [12288, 300] f32 features, idx [12288] i64 (sorted graph ids),
  n () i64 = 32 nodes/graph. 384 graphs.

  out0 X_cat [3840, 300] = concat_g softmax(S_g)^T X_g
  out1 A_bd [3840, 3840] = block_diag_g softmax(S_g)^T A_g softmax(S_g)

Sharding: graphs split across 8 cores (48 graphs each). Pooling is
block-diagonal per graph, so each core only needs its rows of S/X and the
48 diagonal 32x32 blocks of A. Per core, graphs run in 12 groups of 4
(4*32 = 128 nodes = full partition dim):
  - softmax over K=10 for all 1536 rows (one exp + segmented reduce)
  - SD [128, 12*40]: per group a block-diag [128,40] of normalized S
  - BD [128, 12*128]: per group a block-diag [128,128] of A_b^T
  - mm2: T  = BD_g^T @ Sn_g = A_b @ softmax(S_b), stacked    [128,10]
  - mm1: XO = SD_g^T @ X_g                                   [40,300]
  - mm3: AO = SD_g^T @ T_g                                   [40,10]

The host pre-arranges every input into the exact per-core SBUF layout so
each load is one flat 2D DMA (contiguous per partition; no 40-byte
scatter packets), and post-rearranges the device-layout outputs.

Engine/queue routing: sync HWDGE ring = AT + X chunks 0,2 + all stores;
scalar HWDGE ring = S + X chunks 1,3.  DVE runs BD placement before the
softmax reduce chain so the PE can start mm2s early; PSUM evacuation
alternates ACT/DVE.  Matmul dtype is fp32 by default (exact, 2-pass PE);
set KERNEL_F32R=1 for single-pass float32r (~4x faster mm1, ~2e-4 error).
"""

import os
import numpy as np
from contextlib import ExitStack

B = 384        # graphs
NPER = 32      # nodes per graph
K = 10         # clusters
D = 300        # feature dim
NCORES = 8
GPC = B // NCORES          # 48 graphs per core
GRP = 4                    # graphs per 128-row group
NG = GPC // GRP            # 12 groups per core
ROWS = GPC * NPER          # 1536 node rows per core
XCH = 3                    # groups per X input DMA chunk
OCH = 3                    # groups per XO output DMA chunk

_CACHE = {}
LAST_RESULTS = None        # BassKernelResults of the most recent run


def _use_f32r():
    return bool(os.environ.get("KERNEL_F32R"))


def _body(ctx, tc, S_d, X_d, AT_d, XO_d, AO_d):
    import concourse.bass as bass
    import concourse.mybir as mybir

    nc = tc.nc
    f32 = mybir.dt.float32
    mmdt = mybir.dt.float32r if _use_f32r() else f32

    const = ctx.enter_context(tc.tile_pool(name="const", bufs=1))
    psx = ctx.enter_context(tc.tile_pool(name="psx", bufs=3, space="PSUM"))
    psa = ctx.enter_context(tc.tile_pool(name="psa", bufs=2, space="PSUM"))
    pst = ctx.enter_context(tc.tile_pool(name="pst", bufs=2, space="PSUM"))

    S_t = const.tile([128, NG * K], f32, tag="S_t")
    E = const.tile([128, NG * K], f32, tag="E")
    Sn = const.tile([128, NG * K], mmdt, tag="Sn")
    bf16 = mybir.dt.bfloat16
    Snh = const.tile([128, NG * K], bf16, tag="Snh")   # bf16 hi/lo pair of Sn
    Snl = const.tile([128, NG * K], bf16, tag="Snl")
    sums = const.tile([128, NG], f32, tag="sums")
    rinv = const.tile([128, NG], f32, tag="rinv")
    Ast = const.tile([128, NG * NPER], f32, tag="Ast")
    SD = const.tile([128, NG * GRP * K], mmdt, tag="SD")
    BD = const.tile([128, NG * 128], mybir.dt.bfloat16, tag="BD")  # 0/1: exact
    XT = const.tile([128, NG * (D + K)], mmdt, tag="XT")  # [300 X | 10 T] per group
    XOs = const.tile([GRP * K, NG * D], f32, tag="XOs")
    AOs = const.tile([GRP * K, NG * K], f32, tag="AOs")

    # ---- zero-fills on GpSimd (f32 view), off every critical chain ----
    nc.gpsimd.memset(SD[:].bitcast(f32), 0.0)
    nc.gpsimd.memset(BD[:], 0.0)

    # ---- inputs: all flat 2D DMAs (host pre-arranged layouts) ----
    nc.scalar.dma_start(S_t[:], S_d)
    nc.sync.dma_start(Ast[:], AT_d)
    NCH = NG // XCH
    XTv = XT[:].rearrange("p (g m) -> p g m", m=D + K)
    for c in range(NCH):
        (nc.sync if c % 2 == 0 else nc.scalar).dma_start(
            XTv[:, XCH * c:XCH * (c + 1), 0:D],
            X_d[:, XCH * D * c:XCH * D * (c + 1)].rearrange(
                "p (g d) -> p g d", d=D))

    # ---- block-diag A^T placement on DVE first (only needs AT) ----
    A3 = Ast[:].rearrange("p (g q) -> p g q", q=NPER)
    BDv = BD[:].rearrange("p (g m) -> p g m", m=128)
    for b in range(GRP):
        ps = slice(NPER * b, NPER * (b + 1))
        eng = nc.vector.tensor_copy if b % 2 == 0 else nc.scalar.copy
        eng(BDv[ps, :, NPER * b:NPER * (b + 1)], A3[ps, :, :])

    # ---- softmax over K within each group column block ----
    nc.scalar.activation(E[:], S_t[:], mybir.ActivationFunctionType.Exp)
    E3 = E[:].rearrange("p (g k) -> p g k", k=K)
    nc.vector.reduce_sum(sums[:], E3, axis=mybir.AxisListType.X)
    nc.vector.reciprocal(rinv[:], sums[:])
    rb = rinv[:].unsqueeze(2)
    nc.vector.tensor_mul(Sn[:].rearrange("p (g k) -> p g k", k=K), E3,
                         rb.broadcast_to([128, NG, K]))

    nc.vector.tensor_copy(Snh[:], Sn[:])
    nc.vector.tensor_sub(Snl[:], Sn[:], Snh[:])

    # ---- block-diag softmax placement on DVE ----
    Sn3 = Sn[:].rearrange("p (g k) -> p g k", k=K)
    SDv = SD[:].rearrange("p (g m) -> p g m", m=GRP * K)
    for b in range(GRP):
        ps = slice(NPER * b, NPER * (b + 1))
        nc.vector.tensor_copy(SDv[ps, :, K * b:K * (b + 1)], Sn3[ps, :, :])

    # ---- loop A: all mm2 first (PE can start as soon as BD+Sn land) ----
    W = D + K
    for g in range(NG):
        tp = pst.tile([128, K], f32)
        nc.tensor.matmul(tp[:], BD[:, 128 * g:128 * (g + 1)],
                         Snh[:, K * g:K * (g + 1)], start=True, stop=False)
        nc.tensor.matmul(tp[:], BD[:, 128 * g:128 * (g + 1)],
                         Snl[:, K * g:K * (g + 1)], start=False, stop=True)
        nc.scalar.copy(XT[:, W * g + D:W * (g + 1)], tp[:])

    # ---- loop B: merged mm1+mm3 per group; trailing stores kept small ----
    for g in range(NG):
        # PSUM evacuation: late groups pinned (XO->ACT, AO->DVE) so the
        # final copies never queue behind store DMAs on one engine
        if g >= 10:
            cp_xo, cp_ao = nc.scalar.copy, nc.vector.tensor_copy
        elif g % 2 == 0:
            cp_xo, cp_ao = nc.scalar.copy, nc.vector.tensor_copy
        else:
            cp_xo, cp_ao = nc.vector.tensor_copy, nc.scalar.copy

        xo = psx.tile([GRP * K, W], f32)
        nc.tensor.matmul(xo[:], SD[:, GRP * K * g:GRP * K * (g + 1)],
                         XT[:, W * g:W * (g + 1)], start=True, stop=True)
        cp_xo(XOs[:, D * g:D * (g + 1)], xo[:, 0:D])
        cp_ao(AOs[:, K * g:K * (g + 1)], xo[:, D:W])

        # stores: coarse early chunks, fine late ones, all on the sync ring
        # (scalar engine is busy with the trailing copies)
        if g in (2, 5, 8):
            c = g // OCH
            nc.sync.dma_start(XO_d[:, D * OCH * c:D * OCH * (c + 1)],
                              XOs[:, D * OCH * c:D * OCH * (c + 1)])
        elif g >= 9:
            ring = nc.scalar if g == 10 else nc.sync
            ring.dma_start(XO_d[:, D * g:D * (g + 1)],
                           XOs[:, D * g:D * (g + 1)])
        if g in (NG // 2 - 1, NG - 1):
            h = 0 if g == NG // 2 - 1 else 1
            hw = NG // 2 * K
            (nc.scalar if h else nc.sync).dma_start(
                AO_d[:, hw * h:hw * (h + 1)], AOs[:, hw * h:hw * (h + 1)])


def _build():
    key = ("nc", _use_f32r())
    if key in _CACHE:
        return _CACHE[key]
    import concourse.bacc as bacc
    import concourse.tile as tile
    import concourse.mybir as mybir

    f32 = mybir.dt.float32
    mmdt = mybir.dt.float32r if _use_f32r() else f32
    nc = bacc.Bacc("TRN2", target_bir_lowering=False, debug=False)
    # Device-layout tensors (host pre/post-arranges):
    #   S  [128, 120]   col = 10g + k, partition = node p of group g
    #   AT [128, 384]   [32b+q, 32g+p] = A_{4g+b}[p, q]
    #   X  [128, 3600]  col = 300g + d
    #   XO [40, 3600]   row = 10b + i, col = 300g + d  (graph j = 4g+b)
    #   AO [40, 120]    row = 10b + i, col = 10g + k
    S_d = nc.dram_tensor("S", [128, NG * K], f32, kind="ExternalInput").ap()
    X_d = nc.dram_tensor("X", [128, NG * D], mmdt, kind="ExternalInput").ap()
    AT_d = nc.dram_tensor("AT", [128, NG * NPER], f32, kind="ExternalInput").ap()
    XO_d = nc.dram_tensor("XO", [GRP * K, NG * D], f32, kind="ExternalOutput").ap()
    AO_d = nc.dram_tensor("AO", [GRP * K, NG * K], f32, kind="ExternalOutput").ap()

    with tile.TileContext(nc) as tc:
        with ExitStack() as ctx:
            _body(ctx, tc, S_d, X_d, AT_d, XO_d, AO_d)
    nc.compile()
    _CACHE[key] = nc
    return nc


def kernel(S, A, X, idx=None, n=NPER, **_):
    global LAST_RESULTS
    from concourse.bass_utils import run_bass_kernel_spmd

    S = np.asarray(S, dtype=np.float32)
    A = np.asarray(A, dtype=np.float32)
    X = np.asarray(X, dtype=np.float32)
    n = int(np.asarray(n)) if n is not None else NPER
    assert n == NPER and S.shape == (B * NPER, K) and X.shape == (B * NPER, D)

    # Device layouts (see _build).  c = core, g = group, b = graph-in-group.
    S8 = np.ascontiguousarray(
        S.reshape(NCORES, NG, 128, K).transpose(0, 2, 1, 3)
    ).reshape(NCORES, 128, NG * K)
    X8 = np.ascontiguousarray(
        X.reshape(NCORES, NG, 128, D).transpose(0, 2, 1, 3)
    ).reshape(NCORES, 128, NG * D)
    bi = np.arange(B)
    blocks = A.reshape(B, NPER, B, NPER)[bi, :, bi, :]        # [384, 32, 32]
    blocksT = blocks.transpose(0, 2, 1)                       # [j][q, p] = A_j[p, q]
    AT8 = np.ascontiguousarray(
        blocksT.reshape(NCORES, NG, GRP, NPER, NPER).transpose(0, 2, 3, 1, 4)
    ).reshape(NCORES, 128, NG * NPER)

    in_maps = [{"S": S8[c], "X": X8[c], "AT": AT8[c]} for c in range(NCORES)]

    nc = _build()
    kw = {}
    if os.environ.get("KERNEL_TRACE"):
        kw = dict(trace=True, tmpdir=os.environ.get("KERNEL_TRACE_DIR") or None)
    res = run_bass_kernel_spmd(nc, in_maps, list(range(NCORES)), **kw)
    LAST_RESULTS = res

    # XO [40, 3600] -> per-core [12, 40, 300] -> rows 40g+10b+i of X_cat
    X_cat = np.concatenate(
        [r["XO"].reshape(GRP * K, NG, D).transpose(1, 0, 2).reshape(GPC * K, D)
         for r in res.results], axis=0)
    # AO [40, 120] -> blocks [g, b][i, k] -> graph j = 4g+b
    AO = np.stack(
        [r["AO"].reshape(GRP, K, NG, K).transpose(2, 0, 1, 3).reshape(GPC, K, K)
         for r in res.results]).reshape(B, K, K)
    A_bd = np.zeros((B * K, B * K), dtype=np.float32)
    A_bd.reshape(B, K, B, K)[bi, :, bi, :] = AO
    return X_cat, A_bd


# revision 17
# speedup vs baseline: 1.2355x; 1.0014x over previous
"""DiffPool encoder kernel for Trainium2 (Bass/Tile), 8-core SPMD.

Problem (hardcoded shapes):
  S [12288, 10] f32 assignment logits, A [12288, 12288] f32 adjacency,
  X [12288, 300] f32 features, idx [12288] i64 (sorted graph ids),
  n () i64 = 32 nodes/graph. 384 graphs.

  out0 X_cat [3840, 300] = concat_g softmax(S_g)^T X_g
  out1 A_bd [3840, 3840] = block_diag_g softmax(S_g)^T A_g softmax(S_g)

Sharding: graphs split across 8 cores (48 graphs each). Pooling is
block-diagonal per graph, so each core only needs its rows of S/X and the
48 diagonal 32x32 blocks of A. Per core, graphs run in 12 groups of 4
(4*32 = 128 nodes = full partition dim):
  - softmax over K=10 for all 1536 rows (one exp + segmented reduce)
  - SD [128, 12*40]: per group a block-diag [128,40] of normalized S
  - BD [128, 12*128]: per group a block-diag [128,128] of A_b^T
  - mm2: T  = BD_g^T @ Sn_g = A_b @ softmax(S_b), stacked    [128,10]
  - mm1: XO = SD_g^T @ X_g                                   [40,300]
  - mm3: AO = SD_g^T @ T_g                                   [40,10]

The host pre-arranges every input into the exact per-core SBUF layout so
each load is one flat 2D DMA (contiguous per partition; no 40-byte
scatter packets), and post-rearranges the device-layout outputs.

Engine/queue routing: sync HWDGE ring = AT + X chunks 0,2 + all stores;
scalar HWDGE ring = S + X chunks 1,3.  DVE runs BD placement before the
softmax reduce chain so the PE can start mm2s early; PSUM evacuation
alternates ACT/DVE.  Matmul dtype is fp32 by default (exact, 2-pass PE);
set KERNEL_F32R=1 for single-pass float32r (~4x faster mm1, ~2e-4 error).
"""

import os
import numpy as np
from contextlib import ExitStack

B = 384        # graphs
NPER = 32      # nodes per graph
K = 10         # clusters
D = 300        # feature dim
NCORES = 8
GPC = B // NCORES          # 48 graphs per core
GRP = 4                    # graphs per 128-row group
NG = GPC // GRP            # 12 groups per core
ROWS = GPC * NPER          # 1536 node rows per core
XCH = 3                    # groups per X input DMA chunk
OCH = 3                    # groups per XO output DMA chunk

_CACHE = {}
LAST_RESULTS = None        # BassKernelResults of the most recent run


def _use_f32r():
    return bool(os.environ.get("KERNEL_F32R"))


def _body(ctx, tc, S_d, X_d, AT_d, XO_d, AO_d):
    import concourse.bass as bass
    import concourse.mybir as mybir

    nc = tc.nc
    f32 = mybir.dt.float32
    mmdt = mybir.dt.float32r if _use_f32r() else f32

    const = ctx.enter_context(tc.tile_pool(name="const", bufs=1))
    psx = ctx.enter_context(tc.tile_pool(name="psx", bufs=3, space="PSUM"))
    psa = ctx.enter_context(tc.tile_pool(name="psa", bufs=2, space="PSUM"))
    pst = ctx.enter_context(tc.tile_pool(name="pst", bufs=2, space="PSUM"))

    S_t = const.tile([128, NG * K], f32, tag="S_t")
    E = const.tile([128, NG * K], f32, tag="E")
    Sn = const.tile([128, NG * K], mmdt, tag="Sn")
    bf16 = mybir.dt.bfloat16
    Snh = const.tile([128, NG * K], bf16, tag="Snh")   # bf16 hi/lo pair of Sn
    Snl = const.tile([128, NG * K], bf16, tag="Snl")
    sums = const.tile([128, NG], f32, tag="sums")
    rinv = const.tile([128, NG], f32, tag="rinv")
    Ast = const.tile([128, NG * NPER], f32, tag="Ast")
    SD = const.tile([128, NG * GRP * K], mmdt, tag="SD")
    BD = const.tile([128, NG * 128], mybir.dt.bfloat16, tag="BD")  # 0/1: exact
    XT = const.tile([128, NG * (D + K)], mmdt, tag="XT")  # [300 X | 10 T] per group
    XOs = const.tile([GRP * K, NG * D], f32, tag="XOs")
    AOs = const.tile([GRP * K, NG * K], f32, tag="AOs")

    # ---- zero-fills on GpSimd (f32 view), off every critical chain ----
    nc.gpsimd.memset(SD[:].bitcast(f32), 0.0)
    nc.gpsimd.memset(BD[:], 0.0)

    # ---- inputs: all flat 2D DMAs (host pre-arranged layouts) ----
    nc.scalar.dma_start(S_t[:], S_d)
    nc.sync.dma_start(Ast[:], AT_d)
    NCH = NG // XCH
    XTv = XT[:].rearrange("p (g m) -> p g m", m=D + K)
    for c in range(NCH):
        (nc.sync if c % 2 == 0 else nc.scalar).dma_start(
            XTv[:, XCH * c:XCH * (c + 1), 0:D],
            X_d[:, XCH * D * c:XCH * D * (c + 1)].rearrange(
                "p (g d) -> p g d", d=D))

    # ---- block-diag A^T placement on DVE first (only needs AT) ----
    A3 = Ast[:].rearrange("p (g q) -> p g q", q=NPER)
    BDv = BD[:].rearrange("p (g m) -> p g m", m=128)
    for b in range(GRP):
        ps = slice(NPER * b, NPER * (b + 1))
        eng = nc.vector.tensor_copy if b % 2 == 0 else nc.scalar.copy
        eng(BDv[ps, :, NPER * b:NPER * (b + 1)], A3[ps, :, :])

    # ---- softmax over K within each group column block ----
    nc.scalar.activation(E[:], S_t[:], mybir.ActivationFunctionType.Exp)
    E3 = E[:].rearrange("p (g k) -> p g k", k=K)
    nc.vector.reduce_sum(sums[:], E3, axis=mybir.AxisListType.X)
    nc.vector.reciprocal(rinv[:], sums[:])
    rb = rinv[:].unsqueeze(2)
    nc.vector.tensor_mul(Sn[:].rearrange("p (g k) -> p g k", k=K), E3,
                         rb.broadcast_to([128, NG, K]))

    nc.vector.tensor_copy(Snh[:], Sn[:])
    nc.vector.tensor_sub(Snl[:], Sn[:], Snh[:])

    # ---- block-diag softmax placement on DVE ----
    Sn3 = Sn[:].rearrange("p (g k) -> p g k", k=K)
    SDv = SD[:].rearrange("p (g m) -> p g m", m=GRP * K)
    for b in range(GRP):
        ps = slice(NPER * b, NPER * (b + 1))
        nc.vector.tensor_copy(SDv[ps, :, K * b:K * (b + 1)], Sn3[ps, :, :])

    # ---- PE stream: mm2 pairs interleaved with merged mm1+mm3 (lag 3) ----
    W = D + K
    LAG = 3

    def emit_mm2(g):
        tp = pst.tile([128, K], f32)
        nc.tensor.matmul(tp[:], BD[:, 128 * g:128 * (g + 1)],
                         Snh[:, K * g:K * (g + 1)], start=True, stop=False)
        nc.tensor.matmul(tp[:], BD[:, 128 * g:128 * (g + 1)],
                         Snl[:, K * g:K * (g + 1)], start=False, stop=True)
        nc.scalar.copy(XT[:, W * g + D:W * (g + 1)], tp[:])

    def emit_merged(g):
        if g >= 10 or g % 2 == 0:
            cp_xo, cp_ao = nc.scalar.copy, nc.vector.tensor_copy
        else:
            cp_xo, cp_ao = nc.vector.tensor_copy, nc.scalar.copy

        xo = psx.tile([GRP * K, W], f32)
        nc.tensor.matmul(xo[:], SD[:, GRP * K * g:GRP * K * (g + 1)],
                         XT[:, W * g:W * (g + 1)], start=True, stop=True)
        cp_xo(XOs[:, D * g:D * (g + 1)], xo[:, 0:D])
        cp_ao(AOs[:, K * g:K * (g + 1)], xo[:, D:W])

        # stores: coarse early chunks, fine late ones, split across rings
        if g in (2, 5, 8):
            c = g // OCH
            nc.sync.dma_start(XO_d[:, D * OCH * c:D * OCH * (c + 1)],
                              XOs[:, D * OCH * c:D * OCH * (c + 1)])
        elif g >= 9:
            ring = nc.scalar if g == 10 else nc.sync
            ring.dma_start(XO_d[:, D * g:D * (g + 1)],
                           XOs[:, D * g:D * (g + 1)])
        if g in (NG // 2 - 1, NG - 1):
            h = 0 if g == NG // 2 - 1 else 1
            hw = NG // 2 * K
            (nc.scalar if h else nc.sync).dma_start(
                AO_d[:, hw * h:hw * (h + 1)], AOs[:, hw * h:hw * (h + 1)])

    for g in range(NG + LAG):
        if g < NG:
            emit_mm2(g)
        if g >= LAG:
            emit_merged(g - LAG)


def _build():
    key = ("nc", _use_f32r())
    if key in _CACHE:
        return _CACHE[key]
    import concourse.bacc as bacc
    import concourse.tile as tile
    import concourse.mybir as mybir

    f32 = mybir.dt.float32
    mmdt = mybir.dt.float32r if _use_f32r() else f32
    nc = bacc.Bacc("TRN2", target_bir_lowering=False, debug=False)
    # Device-layout tensors (host pre/post-arranges):
    #   S  [128, 120]   col = 10g + k, partition = node p of group g
    #   AT [128, 384]   [32b+q, 32g+p] = A_{4g+b}[p, q]
    #   X  [128, 3600]  col = 300g + d
    #   XO [40, 3600]   row = 10b + i, col = 300g + d  (graph j = 4g+b)
    #   AO [40, 120]    row = 10b + i, col = 10g + k
    S_d = nc.dram_tensor("S", [128, NG * K], f32, kind="ExternalInput").ap()
    X_d = nc.dram_tensor("X", [128, NG * D], mmdt, kind="ExternalInput").ap()
    AT_d = nc.dram_tensor("AT", [128, NG * NPER], f32, kind="ExternalInput").ap()
    XO_d = nc.dram_tensor("XO", [GRP * K, NG * D], f32, kind="ExternalOutput").ap()
    AO_d = nc.dram_tensor("AO", [GRP * K, NG * K], f32, kind="ExternalOutput").ap()

    with tile.TileContext(nc) as tc:
        with ExitStack() as ctx:
            _body(ctx, tc, S_d, X_d, AT_d, XO_d, AO_d)
    nc.compile()
    _CACHE[key] = nc
    return nc


def kernel(S, A, X, idx=None, n=NPER, **_):
    global LAST_RESULTS
    from concourse.bass_utils import run_bass_kernel_spmd

    S = np.asarray(S, dtype=np.float32)
    A = np.asarray(A, dtype=np.float32)
    X = np.asarray(X, dtype=np.float32)
    n = int(np.asarray(n)) if n is not None else NPER
    assert n == NPER and S.shape == (B * NPER, K) and X.shape == (B * NPER, D)

    # Device layouts (see _build).  c = core, g = group, b = graph-in-group.
    S8 = np.ascontiguousarray(
        S.reshape(NCORES, NG, 128, K).transpose(0, 2, 1, 3)
    ).reshape(NCORES, 128, NG * K)
    X8 = np.ascontiguousarray(
        X.reshape(NCORES, NG, 128, D).transpose(0, 2, 1, 3)
    ).reshape(NCORES, 128, NG * D)
    bi = np.arange(B)
    blocks = A.reshape(B, NPER, B, NPER)[bi, :, bi, :]        # [384, 32, 32]
    blocksT = blocks.transpose(0, 2, 1)                       # [j][q, p] = A_j[p, q]
    AT8 = np.ascontiguousarray(
        blocksT.reshape(NCORES, NG, GRP, NPER, NPER).transpose(0, 2, 3, 1, 4)
    ).reshape(NCORES, 128, NG * NPER)

    in_maps = [{"S": S8[c], "X": X8[c], "AT": AT8[c]} for c in range(NCORES)]

    nc = _build()
    kw = {}
    if os.environ.get("KERNEL_TRACE"):
        kw = dict(trace=True, tmpdir=os.environ.get("KERNEL_TRACE_DIR") or None)
    res = run_bass_kernel_spmd(nc, in_maps, list(range(NCORES)), **kw)
    LAST_RESULTS = res

    # XO [40, 3600] -> per-core [12, 40, 300] -> rows 40g+10b+i of X_cat
    X_cat = np.concatenate(
        [r["XO"].reshape(GRP * K, NG, D).transpose(1, 0, 2).reshape(GPC * K, D)
         for r in res.results], axis=0)
    # AO [40, 120] -> blocks [g, b][i, k] -> graph j = 4g+b
    AO = np.stack(
        [r["AO"].reshape(GRP, K, NG, K).transpose(2, 0, 1, 3).reshape(GPC, K, K)
         for r in res.results]).reshape(B, K, K)
    A_bd = np.zeros((B * K, B * K), dtype=np.float32)
    A_bd.reshape(B, K, B, K)[bi, :, bi, :] = AO
    return X_cat, A_bd
